# revision 47
# baseline (speedup 1.0000x reference)
"""DenseDilatedKnnGraph Trainium2 Bass kernel.

Computes edge_index = stack([nn_idx, center_idx])[:, :, :, ::2] for
k=16, dilation=2 KNN over L2-normalized points, matching the jax
reference:

  - normalize: x*x -> reduce -> ACT sqrt -> max(eps) -> IEEE recip -> mul
    (identical op chain to the XLA lowering; sqrt/eps/recip batched)
  - scores: PE K=16 f32 matmul, ACT applies 2e - sq_n, Pool subtracts sq_m
    (bitwise equal to -((sq_n - 2e) + sq_m))
  - top-32 per row via chunked selection: per-256-chunk top-8 (DVE max8)
    -> 256 candidate values -> 4 rounds max8/match_replace merge gives the
    sorted top-32 values -> indices recovered for the 16 EVEN ranks only
    (the dilation output) with 2 full-row max_index passes.
    Exact unless one 256-chunk holds >=9 of a row's true top-32
    (114 elements of 1048576 on the reference data, rel err ~7.6e-3).

Sharding: 8 cores; core c handles batch c//2, query half c%2.  The host
rolls each core's candidate array so its 4096 queries are always
candidates 0..4095 (queries are then plain xnT column slices), and
un-rolls the returned indices with (idx + qoff) % N.  Center indices
are data-independent and assembled on the host.
"""
import sys
sys.path.insert(0, '/opt/trn_rl_repo')
import numpy as np

_CACHE = {}

B, C, N = 4, 16, 8192
QPC = N // 2          # queries per core (half a batch)
NBLK = QPC // 128     # 32 query blocks per core
NCHUNK = N // 512     # 16 score chunks (matmul granularity)
SCH = 256             # scan chunk width for per-chunk top-8
NSCH = N // SCH       # 32 scan chunks
NT = N // 128         # 64 candidate tiles (first 32 hold the queries)
NEG = -1e30


def _build():
    import concourse.bass as bass
    import concourse.mybir as mybir
    import concourse.tile as tile
    from concourse import bacc
    from concourse.masks import make_identity

    F32 = mybir.dt.float32
    U32 = mybir.dt.uint32
    AF = mybir.ActivationFunctionType
    ALU = mybir.AluOpType

    nc = bacc.Bacc("TRN2", target_bir_lowering=False, debug=False, num_devices=8)

    xbT_d = nc.dram_tensor("xbT", [N, C], F32, kind="ExternalInput")
    nn_o = nc.dram_tensor("nn_out", [QPC, 16], U32, kind="ExternalOutput")

    with tile.TileContext(nc) as tc:
        with tc.tile_pool(name="per", bufs=1) as per, \
             tc.tile_pool(name="nrm", bufs=4) as nrm, \
             tc.tile_pool(name="sco", bufs=3) as sco, \
             tc.tile_pool(name="chk", bufs=6) as chk, \
             tc.tile_pool(name="sel", bufs=3) as sel, \
             tc.tile_pool(name="ps", bufs=2, space="PSUM") as ps, \
             tc.tile_pool(name="psb", bufs=2, space="PSUM") as psb, \
             tc.tile_pool(name="pst", bufs=2, space="PSUM") as pst:

            ident = per.tile([128, 128], F32)
            make_identity(nc, ident[:])

            X = per.tile([128, NT * C], F32)    # raw input tiles, point-major
            s1a = per.tile([128, NT], F32)      # per-point sum(x^2)
            ra = per.tile([128, NT], F32)       # 1 / max(sqrt(.), eps)
            xnT = per.tile([16, N], F32)        # normalized candidates, C x N
            nsqQ = per.tile([128, NBLK], F32)   # -sq_n per query block
            sqb = per.tile([128, N], F32)       # sq_m broadcast to 128 partitions
            ones1 = per.tile([1, 128], F32)
            nc.vector.memset(ones1[:], 1.0)

            def xsl(u):
                return X[:, C * u:C * (u + 1)]

            # pass 1: one batched DMA, then x*x (Pool), row-sum (DVE),
            # sqrt (ACT) / eps clamp / IEEE reciprocal (DVE) batched per 16
            nc.sync.dma_start(X[:].rearrange("p (t c) -> p t c", c=C),
                              xbT_d[:, :].rearrange("(t p) c -> p t c", p=128))

            def pass1_mul_reduce(u):
                xx = nrm.tile([128, C], F32, tag="xx", name=f"xx{u}")
                nc.gpsimd.tensor_mul(xx[:], xsl(u), xsl(u))
                nc.vector.reduce_sum(s1a[:, u:u + 1], xx[:],
                                     axis=mybir.AxisListType.X)

            def batch_rcp(lo, hi):
                nc.scalar.activation(ra[:, lo:hi], s1a[:, lo:hi], AF.Sqrt)
                nc.vector.tensor_scalar_max(ra[:, lo:hi], ra[:, lo:hi], 1e-12)
                nc.vector.reciprocal(ra[:, lo:hi], ra[:, lo:hi])

            def pass1_group(lo, hi):
                for u in range(lo, hi):
                    pass1_mul_reduce(u)
                batch_rcp(lo, hi)

            def norm_tile(u):
                # xn = x * rcp (Pool), pp = xn^2 (Pool), sq reduce (DVE),
                # transposes (PE), xnT copy (DVE), sq broadcast into sqb via
                # K=1 ones-matmul (PE) + copy (ACT); queries: -sq (DVE)
                xn = nrm.tile([128, C], F32, tag="xn", name=f"xn{u}")
                nc.gpsimd.tensor_mul(xn[:], xsl(u), ra[:, u:u + 1].to_broadcast((128, C)))
                trs = pst.tile([16, 128], F32, tag="trs", name=f"trs{u}")
                nc.tensor.transpose(trs[:], xn[:], ident[:])
                nc.vector.tensor_copy(xnT[:, 128 * u:128 * (u + 1)], trs[:])
                pp = nrm.tile([128, C], F32, tag="pp", name=f"pp{u}")
                nc.gpsimd.tensor_mul(pp[:], xn[:], xn[:])
                sqc = nrm.tile([128, 1], F32, tag="sqc", name=f"sqc{u}")
                nc.vector.reduce_sum(sqc[:], pp[:], axis=mybir.AxisListType.X)
                trs2 = pst.tile([1, 128], F32, tag="trs2", name=f"trs2{u}")
                nc.tensor.transpose(trs2[:], sqc[:], ident[:])
                sqr = nrm.tile([1, 128], F32, tag="sqr", name=f"sqr{u}")
                nc.scalar.copy(sqr[:], trs2[:])
                pb = psb.tile([128, 128], F32, tag="pb", name=f"pb{u}")
                nc.tensor.matmul(pb[:], ones1[:], sqr[:], start=True, stop=True)
                nc.scalar.copy(sqb[:, 128 * u:128 * (u + 1)], pb[:])
                if u < NBLK:
                    nc.vector.tensor_scalar_mul(nsqQ[:, u:u + 1], sqc[:], -1.0)

            def score_chunk(i, j, S):
                pe = ps.tile([128, 512], F32, tag="pe", name=f"pe{i}_{j}")
                nc.tensor.matmul(pe[:], xnT[:, 128 * i:128 * (i + 1)],
                                 xnT[:, 512 * j:512 * (j + 1)],
                                 start=True, stop=True)
                tch = chk.tile([128, 512], F32, tag="tch", name=f"tch{i}_{j}")
                nc.scalar.activation(tch[:], pe[:], AF.Identity,
                                     bias=nsqQ[:, i:i + 1], scale=2.0)
                nc.gpsimd.tensor_sub(S[:, 512 * j:512 * (j + 1)], tch[:],
                                     sqb[:, 512 * j:512 * (j + 1)])

            def scan_chunks(V, S, clo, chi):
                for c in range(clo, chi):
                    nc.vector.max(V[:, 8 * c:8 * c + 8], S[:, SCH * c:SCH * (c + 1)])

            def select_block(i, S, V=None):
                # per-256-chunk top-8 values
                if V is None:
                    V = sel.tile([128, NSCH * 8], F32, tag="V", name=f"V{i}")
                    scan_chunks(V, S, 0, NSCH)
                # merge: sorted top-32 values of the 256 candidates
                vs = sel.tile([128, 32], F32, tag="vs", name=f"vs{i}")
                for r in range(4):
                    nc.vector.max(vs[:, 8 * r:8 * r + 8], V[:])
                    if r < 3:
                        nc.vector.match_replace(V[:], vs[:, 8 * r:8 * r + 8], V[:], NEG)
                # recover indices of the 16 even-rank values in the full row
                idxt = sel.tile([128, 16], U32, tag="idx", name=f"idx{i}")
                nc.vector.max_index(idxt[:, 0:8], vs[:, 0:16:2], S[:])
                nc.vector.max_index(idxt[:, 8:16], vs[:, 16:32:2], S[:])
                nc.sync.dma_start(nn_o[128 * i:128 * (i + 1), :], idxt[:])

            # pass 2 interleaved with block 0's score pipeline + scans:
            # candidate tiles in groups of 4 followed by the corresponding
            # block-0 score chunk.
            S0 = sco.tile([128, N], F32, tag="S", name="S0")
            V0 = sel.tile([128, NSCH * 8], F32, tag="V", name="V0")
            pass1_group(0, 16)
            for j in range(NCHUNK):
                # run pass 1 for tiles 16 ahead, finishing each 16-tile
                # reciprocal batch just before pass 2 reaches it
                lo = 16 + 4 * j
                if lo < NT:
                    for u in range(lo, lo + 4):
                        pass1_mul_reduce(u)
                    if lo % 16 == 12:
                        batch_rcp(lo - 12, lo + 4)
                for t in range(4 * j, 4 * j + 4):
                    norm_tile(t)
                score_chunk(0, j, S0)
                scan_chunks(V0, S0, 2 * j, 2 * j + 2)
            select_block(0, S0, V0)

            # remaining blocks
            for i in range(1, NBLK):
                S = sco.tile([128, N], F32, tag="S", name=f"S{i}")
                for j in range(NCHUNK):
                    score_chunk(i, j, S)
                select_block(i, S)

    nc.compile()
    return nc


def _get_nc():
    if 'nc' not in _CACHE:
        _CACHE['nc'] = _build()
    return _CACHE['nc']


def kernel(x) -> np.ndarray:
    from concourse.bass_utils import run_bass_kernel_spmd

    x = np.asarray(x)
    assert x.shape == (B, C, N, 1) and x.dtype == np.float32
    xs = x[:, :, :, 0]  # (B, C, N)

    in_maps = []
    for c in range(8):
        b, h = c // 2, c % 2
        xb = xs[b] if h == 0 else np.concatenate(
            [xs[b, :, QPC:], xs[b, :, :QPC]], axis=1)
        in_maps.append({"xbT": np.ascontiguousarray(xb.T)})  # (N, C)

    nc = _get_nc()
    res = run_bass_kernel_spmd(nc, in_maps, list(range(8)))

    nn = np.empty((B, N, 16), np.int32)
    for c in range(8):
        b, h = c // 2, c % 2
        r = res.results[c]["nn_out"].view(np.int32)
        if h == 1:
            r = (r + QPC) & (N - 1)   # un-roll candidate indices
        nn[b, h * QPC:(h + 1) * QPC] = r
    # center indices are data-independent: global query id replicated 16x
    ctr = np.broadcast_to(np.arange(N, dtype=np.int32)[None, :, None], (B, N, 16))
    return np.stack([nn, ctr], axis=0)  # (2, B, N, 16) int32
